# revision 27
# baseline (speedup 1.0000x reference)
"""Multi-head causal self-attention (B=2, S=2048, E=1024, H=16) on 8 TRN2 NeuronCores.

Sharding: tensor-parallel over heads (2 heads/core, both batches). Per core:
  - QKV projection for its 2 heads (q^T/k^T transposed layout, v natural;
    both biases are folded away: softmax rows sum to 1, so bv rides
    through the output projection as bo' = bo + bv@W_o, computed host-side)
  - causal flash-style attention, scores computed transposed (k on
    partitions) and exp'd on ACT in 1024-wide PSUM tiles; tri-mask on DVE
  - PV computed FLIPPED (lhsT = [v | ones], rhs = exp(scores^T)): each
    matmul streams 512 q-columns (matmul-bound, no LDWEIGHTS stalls) and
    the result lands pre-transposed [d, q] with the softmax denominator in
    row 64; a K=1 matmul broadcasts the denominator row across 64
    partitions, reciprocal_approx_fast inverts the broadcast, one DVE
    multiply normalizes into per-head attnT halves (all partition-aligned)
  - one 512KB AllToAll per batch, triggered from the otherwise-empty
    GpSimd queue the moment that batch's attnT completes (collectives
    have a ~13us fixed cost, so fewer, larger A2As beat four half-sized
    ones), then four output projections (bias added by the DVE copyout
    against a host-broadcast bo'), each overlapping the next collective.
Emission is phase-sequential (matching HAM clock-gate behavior: dense PE
phases stay at 8/8): qkv0+scores0 (reversed, paced), qkv1+scores1
(middle-out: high k-blocks paced with the suffix-first chunks), then the
batch-1 low score blocks INTERLEAVED with pv0 (pv0's exps are long done,
so its chains fill the PE while scores1lo is ACT-bound, and A2A(0) fires
~18us earlier -- early enough that oproj(0) never waits on it), pv1,
A2A(1), oproj x4 (bf16 out DMAs). Startup interleaves wqk/x chunk-0 loads
per e-block so the first matmul starts after ~1/8 of the startup bytes.
All 16 exp tiles per (batch, head) keep 4 slot buffers so batch-1 exps
never wait on batch-0's PV to release them. Host side only reshapes/
slices inputs and concatenates the 8 disjoint row shards of the output.
"""

import numpy as np
import ml_dtypes

P = 128
B, S, E, H, D = 2, 2048, 1024, 16, 64
NCORES = 8
EB = E // P            # 8 e-blocks
BS = B * S             # 4096 flattened rows
SBB = S // P           # 16 s-blocks per batch
SB = BS // P           # 32 s-blocks global
HPC = H // NCORES      # 2 heads per core
CH = BS // NCORES      # 512 rows owned per core
QT = 512               # q-tile width for the PV phase
NQT = S // QT          # 4 q-tiles per batch

_bf16 = ml_dtypes.bfloat16
_cache = {}


def _build(no_cc=False):
    from contextlib import ExitStack

    import concourse.tile as tile
    from concourse import bacc, mybir

    bf16 = mybir.dt.bfloat16
    f32 = mybir.dt.float32

    nc = bacc.Bacc("TRN2", target_bir_lowering=False, debug=False,
                   num_devices=NCORES)

    # host-side layouts are [partition, eblock, col] so each load is one DMA
    xT_d = nc.dram_tensor("xT", [P, EB, BS], bf16, kind="ExternalInput")
    wqk_d = nc.dram_tensor("wqk", [P, EB, 2 * P], bf16, kind="ExternalInput")
    wv_d = nc.dram_tensor("wv", [P, EB, P], bf16, kind="ExternalInput")
    wo_d = nc.dram_tensor("wo", [P, EB, E], bf16, kind="ExternalInput")
    bqk_d = nc.dram_tensor("bqk", [P, 2], f32, kind="ExternalInput")
    bo_d = nc.dram_tensor("bo", [P, E], bf16, kind="ExternalInput")
    tri_d = nc.dram_tensor("tri", [P, P], bf16, kind="ExternalInput")
    # rank r owns interleaved token blocks {r, r+8, r+16, r+24}: one AllToAll
    # per batch. out row-block st <-> global block st*8 + rank.
    out_d = nc.dram_tensor("out", [4, P, E], bf16, kind="ExternalOutput")
    a2a_in = [nc.dram_tensor(f"a2a_in{b}", [NCORES, P, 2 * P], bf16)
              for b in range(B)]
    a2a_out = [nc.dram_tensor(f"a2a_out{b}", [NCORES, P, 2 * P], bf16)
               for b in range(B)]

    with tile.TileContext(nc) as tc, ExitStack() as ctx:
        consts = ctx.enter_context(tc.tile_pool(name="consts", bufs=1))
        work = ctx.enter_context(tc.tile_pool(name="work", bufs=1))
        xpool = ctx.enter_context(tc.tile_pool(name="xstream", bufs=2))
        epool = ctx.enter_context(tc.tile_pool(name="expst", bufs=2))
        small = ctx.enter_context(tc.tile_pool(name="small", bufs=2))
        opool = ctx.enter_context(tc.tile_pool(name="osb", bufs=1))
        pbig = ctx.enter_context(tc.tile_pool(name="pbig", bufs=2, space="PSUM"))
        ppv = ctx.enter_context(tc.tile_pool(name="ppv", bufs=1, space="PSUM"))
        psm = ctx.enter_context(tc.tile_pool(name="psm", bufs=2, space="PSUM"))

        wqk = consts.tile([P, EB, 2 * P], bf16, tag="wqk")
        wv = consts.tile([P, EB, P], bf16, tag="wv")
        bqk = consts.tile([P, 2], f32, tag="bqk")
        bo = consts.tile([P, E], bf16, tag="bo")
        tri = consts.tile([P, P], bf16, tag="tri")
        ones1 = consts.tile([1, P], bf16, tag="ones1")
        # all-ones column block: row 64 serves as the K=1 stationary
        # operand that broadcasts the denominator row (also at partition 64)
        onesc = consts.tile([P, D], bf16, tag="onesc")

        nc.vector.memset(ones1[:1, :], 1.0)
        nc.vector.memset(onesc[:], 1.0)

        qkT = [work.tile([P, 2, S], bf16, tag=f"qkT{b}", name=f"qkT{b}")
               for b in range(B)]
        vsb = [work.tile([P, SBB, HPC, 66], bf16, tag=f"vsb{b}", name=f"vsb{b}")
               for b in range(B)]
        # per-head attnT halves (both on partitions 0-63): keeps every DVE
        # normalize op partition-aligned; the bounce DMA does the shift of
        # head 1 into partitions 64-127 of the A2A payload
        attnT = [[work.tile([D, S], bf16, tag=f"attnT{h}",
                            name=f"attnT{b}{h}") for h in range(HPC)]
                 for b in range(B)]

        def qkv_pieces(b):
            """QKV projection for batch b, one 512-token chunk per piece.

            Chunks are emitted suffix-first: causal score block kb only needs
            token columns >= kb*128, so late chunks unblock the small k-blocks
            early and ACT (exp) can start before the whole projection is done.
            """
            nc.vector.memset(vsb[b][:], 1.0)
            for i, sc in enumerate(reversed(range(S // 512))):
                gc = b * S + sc * 512  # global col
                xc = xpool.tile([P, EB, 512], bf16, tag="xc", name="xc")
                if b == 0 and i == 0:
                    # interleave wqk/x per-eb so matmul eb=0 can start
                    # after ~1/8 of the startup bytes; remaining consts
                    # queue behind it off the critical path
                    for eb in range(EB):
                        nc.sync.dma_start(wqk[:, eb, :], wqk_d[:, eb, :])
                        nc.sync.dma_start(xc[:, eb, :],
                                          xT_d[:, eb, gc:gc + 512])
                        if eb == 0:
                            nc.sync.dma_start(bqk[:], bqk_d[:, :])
                    nc.sync.dma_start(wv[:], wv_d[:, :, :])
                    nc.sync.dma_start(tri[:], tri_d[:, :])
                else:
                    nc.sync.dma_start(xc[:], xT_d[:, :, gc:gc + 512])
                for db in range(2):
                    ps = psm.tile([P, 512], f32, tag="mid", name="psqk")
                    for eb in range(EB):
                        nc.tensor.matmul(
                            ps[:],
                            lhsT=wqk[:, eb, db * P:(db + 1) * P],
                            rhs=xc[:, eb, :],
                            start=(eb == 0), stop=(eb == EB - 1),
                        )
                    nc.vector.tensor_scalar_add(
                        qkT[b][:, db, sc * 512:(sc + 1) * 512], ps[:],
                        bqk[:, db:db + 1])
                    yield
                for si in range(4):
                    sb = sc * 4 + si
                    pv_ = psm.tile([P, P], f32, tag="mid", name="psv")
                    for eb in range(EB):
                        nc.tensor.matmul(
                            pv_[:], lhsT=xc[:, eb, si * P:(si + 1) * P],
                            rhs=wv[:, eb, :], start=(eb == 0),
                            stop=(eb == EB - 1))
                    # v bias is NOT added here: softmax rows sum to 1, so
                    # bv@W_o folds into b_o host-side (exact); one 3D-AP
                    # copy drops both heads' slices in place
                    nc.vector.tensor_copy(
                        vsb[b][:, sb, :, 0:64],
                        pv_[:].rearrange("p (h d) -> p h d", h=2))
                    yield

        def score_pieces(b, h, expst, order=None):
            """scores^T + exp for one (batch, head), one k-block per piece.

            Default k-block order is high-to-low, matching qkv_pieces'
            suffix-first chunks. Batch 1 uses middle-out ([15..8, 0..7]) so
            its PV q-tiles unlock incrementally as the low k-blocks arrive.
            """
            hs = slice(h * 64, (h + 1) * 64)
            if not expst:
                expst.extend([None] * SBB)
            if order is None:
                order = list(reversed(range(SBB)))
            for kb in order:
                L = S - kb * P
                # 4 bufs: both batches' tiles live concurrently, so
                # batch-1 exps never wait on batch-0's PV to release slots
                et = epool.tile([P, L], bf16, tag=f"e{kb}", name=f"e{kb}",
                                bufs=4)
                off = kb * P
                pos = 0
                while pos < L:  # 1024-wide psum tiles: 1 exp op per tile
                    c = min(1024, L - pos)
                    ps = pbig.tile([P, 1024], f32, tag="big", name="pssc")
                    for c0 in range(0, c, 512):
                        w = min(512, c - c0)
                        nc.tensor.matmul(
                            ps[:, c0:c0 + w],
                            lhsT=qkT[b][hs, 1, off:off + P],
                            rhs=qkT[b][hs, 0, off + pos + c0:off + pos + c0 + w],
                            start=True, stop=True)
                    nc.scalar.activation(
                        et[:, pos:pos + c], ps[:, :c],
                        mybir.ActivationFunctionType.Exp)
                    pos += c
                # zero the invalid (q < k) half of the diagonal block.
                # DVE (not GpSimd): keeps the gpsimd queue empty so the
                # collective triggers fire as soon as their DMAs land.
                nc.vector.tensor_mul(et[:, 0:P], et[:, 0:P], tri[:])
                expst[kb] = et
                yield

        def pv_pieces(b, e0, e1):
            """Flipped PV for batch b: one (q-tile, head) chain per piece.

            out[d, q] = sum_kb vsb[kb]^T @ expst[kb][:, qwin]: N=512 moving
            columns per matmul, stationary operand only 65 columns, so the
            PE stays matmul-bound (no LDWEIGHTS stalls, no HAM cooldown).
            Row 64 accumulates the softmax denominator (ones column of vsb).
            After both heads' chains for a q-tile: reciprocal rows ->
            K=2 broadcast matmul -> two DVE mults write attnT normalized.
            """
            expst = (e0, e1)
            for qt in range(NQT):
                q0 = qt * QT
                pvs = [None, None]
                for h in range(HPC):
                    pp = ppv.tile([65, QT], f32, tag=f"pv{h}",
                                  name=f"pv{h}")
                    nkb = 4 * qt + 4  # k-blocks touching this q-tile
                    for kb in range(nkb):
                        ecol = q0 - kb * P  # expst col of q-tile start
                        poff = max(0, -ecol)
                        w = QT - poff
                        nc.tensor.matmul(
                            pp[:, poff:QT],
                            lhsT=vsb[b][:, kb, h, 0:65],
                            rhs=expst[h][kb][:, ecol + poff:ecol + poff + w],
                            start=(kb == 0), stop=(kb == nkb - 1))
                    # fast-release: one DVE copy frees the PSUM slot so the
                    # next chain never waits on the normalize tail
                    pvs[h] = small.tile([65, QT], bf16, tag=f"pvs{h}",
                                        name=f"pvs{h}", bufs=1)
                    nc.vector.tensor_copy(pvs[h][:], pp[:, :])
                    yield
                # broadcast each raw denominator row across 64 partitions
                # with a K=1 matmul, take the reciprocal on the broadcast
                # (per-lane cost is free-dim-bound, so this costs the same
                # as a single-row reciprocal but needs no extra copy), then
                # normalize into the head's attnT half
                for h in range(HPC):
                    bc = psm.tile([D, QT], f32, tag="mid", name="bc")
                    nc.tensor.matmul(bc[0:D, :],
                                     lhsT=onesc[64:65, 0:D],
                                     rhs=pvs[h][64:65, :],
                                     start=True, stop=True)
                    bcs = small.tile([D, QT], f32, tag="bcs",
                                     name=f"bcs{h}", bufs=1)
                    nc.vector.reciprocal_approx_fast(out=bcs[:],
                                                     in_=bc[0:D, :])
                    nc.vector.tensor_mul(attnT[b][h][0:D, q0:q0 + QT],
                                         pvs[h][0:D, :], bcs[0:D, :])
                yield

        def interleave(*gens):
            gens = list(gens)
            while gens:
                gens = [g for g in gens if next(g, StopIteration) is not StopIteration]

        def paced(qg, score_gens, pv_gens=(), pv_every=1):
            """Weave one qkv stream with score/pv streams, pacing emission so
            every score k-block is emitted AFTER the qkv chunk that writes the
            qkT columns it reads (Tile only tracks writer->reader deps in
            emission order). qkv chunk g (suffix-first) unlocks score k-blocks
            [12-4g, 15-4g]."""
            rnd = 0
            for g in range(4):
                for _ in range(6):
                    next(qg, None)
                for _ in range(4):
                    for sg in score_gens:
                        next(sg, None)
                    if rnd % pv_every == 0:
                        for pg in pv_gens:
                            next(pg, None)
                    rnd += 1
            interleave(qg, *score_gens, *pv_gens)

        atf = [work.tile([P, EB, 2 * P], bf16, tag="atf",
                         name=f"atf{b}") for b in range(B)]

        def bounce(b):
            """attnT -> a2a_in: chunk j of the bounce gets token blocks
            {j, j+8}; head h's 64 rows land at payload partitions h*64+.
            On the otherwise-empty GpSimd SWDGE queue so the collective
            trigger right behind it fires immediately."""
            for t in range(2):
                for h in range(HPC):
                    nc.gpsimd.dma_start(
                        a2a_in[b].ap().rearrange(
                            "j p (t c) -> p j t c",
                            t=2)[h * D:(h + 1) * D, :, t, :],
                        attnT[b][h][:, t * NCORES * P:(t + 1) * NCORES * P]
                        .rearrange("p (j c) -> p j c", c=P))

        def a2a_batch(b):
            """AllToAll of batch b (512KB per rank)."""
            if no_cc:
                for j in range(NCORES):
                    nc.sync.dma_start(a2a_out[b][j], a2a_in[b][j])
            else:
                nc.gpsimd.collective_compute(
                    "AllToAll", mybir.AluOpType.bypass,
                    replica_groups=[list(range(NCORES))],
                    ins=[a2a_in[b].ap()], outs=[a2a_out[b].ap()])

        def atf_gather(b):
            nc.sync.dma_start(
                atf[b][:, :, :],
                a2a_out[b].ap().rearrange("j p c -> p j c"))

        def oproj_half(b, st):
            """Output projection of token block st*8 + rank of batch b.
            b_o is host-broadcast to all partitions, so the bias rides the
            PSUM->SBUF copyout as a DVE add (no K=1 bias matmuls)."""
            ot = opool.tile([P, E], bf16, tag="o", name="ot")
            po = pbig.tile([P, 1024], f32, tag="big", name="pso")
            for oh in range(2):
                for eb in range(EB):
                    nc.tensor.matmul(
                        po[:, oh * 512:(oh + 1) * 512],
                        lhsT=atf[b][:, eb, st * P:(st + 1) * P],
                        rhs=woh[oh][:, eb, :],
                        start=(eb == 0), stop=(eb == EB - 1))
            nc.vector.tensor_tensor(out=ot[:], in0=po[:], in1=bo[:],
                                    op=mybir.AluOpType.add)
            nc.sync.dma_start(out_d[b * 2 + st], ot[:])

        # ---- pipelined emission (priorities; Tile schedules by readiness) ----
        # Phase-sequential PE stream (in-order engine queues make fine
        # interleaving counterproductive): batch-0 QKV+scores, batch-1
        # QKV+scores (ACT exps trail), then both PV phases back to back --
        # each triggers its half-AllToAlls as attnT halves complete -- and
        # the four output projections last, overlapping the tail collectives.
        e00, e01, e10, e11 = [], [], [], []
        paced(qkv_pieces(0),
              [score_pieces(0, 0, e00), score_pieces(0, 1, e01)])
        nc.sync.dma_start(bo[:, :], bo_d[:, :])
        # batch 1 middle-out: high k-blocks pace with the suffix-first qkv
        # chunks; the low half is emitted only after ALL qkv1 pieces (its
        # matmuls read every qkT column -- emission order must respect
        # writer->reader) and runs forward so pv(1) unlocks incrementally.
        # k-blocks 7..4 need only qkv1 chunks sc>=1 (stationary k-cols
        # 512-1023, moving q-cols >= 512), so they pace inside phase 2's
        # rounds; only kb 0-3 must trail the final chunk -- this pulls
        # ~10us of exp off the post-projection critical path
        mid_hi = list(reversed(range(8, SBB))) + [7, 6, 5, 4]
        mid_lo = list(range(4))
        # PV(0) rides phase 2's pacing (its exps are all phase-1 output),
        # so A2A(0) triggers right at qkv1's end -- early enough to clear
        # the serial CC stream before A2A(1) is ready, which was the
        # binding tail edge (A2A(1) blocked ~19us behind A2A(0)).
        paced(qkv_pieces(1),
              [score_pieces(1, 0, e10, mid_hi),
               score_pieces(1, 1, e11, mid_hi)],
              pv_gens=[pv_pieces(0, e00, e01)])
        # W_o halves land in the two xc slots the moment QKV stops using
        # them (same shape/tag); loaded well before the first oproj
        woh = [xpool.tile([P, EB, 512], bf16, tag="xc", name=f"wo{oh}")
               for oh in range(2)]
        for oh in range(2):
            nc.sync.dma_start(woh[oh][:], wo_d[:, :, oh * 512:(oh + 1) * 512])
        bounce(0)
        a2a_batch(0)            # overlaps scores1lo + batch-1 PV
        interleave(score_pieces(1, 0, e10, mid_lo),
                   score_pieces(1, 1, e11, mid_lo))
        interleave(pv_pieces(1, e10, e11))
        bounce(1)
        a2a_batch(1)            # overlaps oproj of batch 0
        atf_gather(0)
        atf_gather(1)
        oproj_half(0, 0)
        oproj_half(0, 1)
        oproj_half(1, 0)
        oproj_half(1, 1)

    nc.compile()
    return nc


def _in_maps(x, W_qkv, b_qkv, W_o, b_o):
    # [partition, eblock, col] layouts (see dram tensor decls)
    xT = np.ascontiguousarray(
        x.reshape(BS, EB, P).transpose(2, 1, 0)).astype(_bf16)
    wo = np.ascontiguousarray(
        W_o.reshape(EB, P, E).transpose(1, 0, 2)).astype(_bf16)
    # fold the v bias through the output projection: softmax rows sum to
    # 1, so attn = softmax@v + bv and out = softmax@v@W_o + (bv@W_o + b_o)
    bo2 = np.asarray(b_o, np.float64) + np.asarray(
        b_qkv[2 * E:], np.float64) @ np.asarray(W_o, np.float64)
    bo = np.ascontiguousarray(np.broadcast_to(
        bo2.reshape(1, E), (P, E))).astype(_bf16)
    tri = np.triu(np.ones((P, P), np.float32)).astype(_bf16)
    maps = []
    for c in range(NCORES):
        o = c * HPC * D
        q_sl = slice(o, o + HPC * D)
        k_sl = slice(E + o, E + o + HPC * D)
        v_sl = slice(2 * E + o, 2 * E + o + HPC * D)
        wqk = np.concatenate(
            [W_qkv[:, q_sl] * 0.125, W_qkv[:, k_sl]], axis=1)
        maps.append({
            "xT": xT,
            "wqk": np.ascontiguousarray(
                wqk.reshape(EB, P, 2 * P).transpose(1, 0, 2)).astype(_bf16),
            "wv": np.ascontiguousarray(
                W_qkv[:, v_sl].reshape(EB, P, P).transpose(1, 0, 2)).astype(_bf16),
            "wo": wo,
            "bqk": np.stack([b_qkv[q_sl] * 0.125,
                             b_qkv[k_sl]], axis=1).astype(np.float32),
            "bo": bo,
            "tri": tri,
        })
    return maps


def kernel(x, W_qkv, b_qkv, W_o, b_o, mask):
    from concourse.bass_utils import run_bass_kernel_spmd

    if "nc" not in _cache:
        _cache["nc"] = _build()
    nc = _cache["nc"]
    maps = _in_maps(np.asarray(x, np.float32), np.asarray(W_qkv, np.float32),
                    np.asarray(b_qkv, np.float32), np.asarray(W_o, np.float32),
                    np.asarray(b_o, np.float32))
    res = run_bass_kernel_spmd(nc, maps, list(range(NCORES)))
    # rank r's out[st] is global 128-token block st*8 + r
    full = np.empty((SB, P, E), np.float32)
    for r in range(NCORES):
        full[r::NCORES] = res.results[r]["out"]
    return full.reshape(B, S, E).astype(np.float32)



# revision 29
# speedup vs baseline: 1.0456x; 1.0456x over previous
"""Multi-head causal self-attention (B=2, S=2048, E=1024, H=16) on 8 TRN2 NeuronCores.

Sharding: tensor-parallel over heads (2 heads/core, both batches). Per core:
  - QKV projection for its 2 heads (q^T/k^T transposed layout, v natural;
    both biases are folded away: softmax rows sum to 1, so bv rides
    through the output projection as bo' = bo + bv@W_o, computed host-side)
  - causal flash-style attention, scores computed transposed (k on
    partitions) and exp'd on ACT in 1024-wide PSUM tiles; tri-mask on DVE
  - PV computed FLIPPED (lhsT = [v | ones], rhs = exp(scores^T)): each
    matmul streams 512 q-columns (matmul-bound, no LDWEIGHTS stalls) and
    the result lands pre-transposed [d, q] with the softmax denominator in
    row 64; a K=1 matmul broadcasts the denominator row across 64
    partitions, reciprocal_approx_fast inverts the broadcast, one DVE
    multiply normalizes into per-head attnT halves (all partition-aligned)
  - one 512KB AllToAll per batch, triggered from the otherwise-empty
    GpSimd queue the moment that batch's attnT completes (collectives
    have a ~13us fixed cost, so fewer, larger A2As beat four half-sized
    ones), then four output projections (bias added by the DVE copyout
    against a host-broadcast bo'), each overlapping the next collective.
Emission is phase-sequential (matching HAM clock-gate behavior: dense PE
phases stay at 8/8): qkv0+scores0 (reversed, paced), qkv1+scores1
(middle-out: high k-blocks paced with the suffix-first chunks), then the
batch-1 low score blocks INTERLEAVED with pv0 (pv0's exps are long done,
so its chains fill the PE while scores1lo is ACT-bound, and A2A(0) fires
~18us earlier -- early enough that oproj(0) never waits on it), pv1,
A2A(1), oproj x4 (bf16 out DMAs). Startup interleaves wqk/x chunk-0 loads
per e-block so the first matmul starts after ~1/8 of the startup bytes.
All 16 exp tiles per (batch, head) keep 4 slot buffers so batch-1 exps
never wait on batch-0's PV to release them. Host side only reshapes/
slices inputs and concatenates the 8 disjoint row shards of the output.
"""

import numpy as np
import ml_dtypes

P = 128
B, S, E, H, D = 2, 2048, 1024, 16, 64
NCORES = 8
EB = E // P            # 8 e-blocks
BS = B * S             # 4096 flattened rows
SBB = S // P           # 16 s-blocks per batch
SB = BS // P           # 32 s-blocks global
HPC = H // NCORES      # 2 heads per core
CH = BS // NCORES      # 512 rows owned per core
QT = 512               # q-tile width for the PV phase
NQT = S // QT          # 4 q-tiles per batch

_bf16 = ml_dtypes.bfloat16
_cache = {}


def _build(no_cc=False):
    from contextlib import ExitStack

    import concourse.tile as tile
    from concourse import bacc, mybir

    bf16 = mybir.dt.bfloat16
    f32 = mybir.dt.float32

    nc = bacc.Bacc("TRN2", target_bir_lowering=False, debug=False,
                   num_devices=NCORES)

    # host-side layouts are [partition, eblock, col] so each load is one DMA
    xT_d = nc.dram_tensor("xT", [P, EB, BS], bf16, kind="ExternalInput")
    wqk_d = nc.dram_tensor("wqk", [P, EB, 2 * P], bf16, kind="ExternalInput")
    wv_d = nc.dram_tensor("wv", [P, EB, P], bf16, kind="ExternalInput")
    wo_d = nc.dram_tensor("wo", [P, EB, E], bf16, kind="ExternalInput")
    bqk_d = nc.dram_tensor("bqk", [P, 2], f32, kind="ExternalInput")
    bo_d = nc.dram_tensor("bo", [P, E], bf16, kind="ExternalInput")
    tri_d = nc.dram_tensor("tri", [P, P], bf16, kind="ExternalInput")
    # rank r owns interleaved token blocks {r, r+8, r+16, r+24}: one AllToAll
    # per batch. out row-block st <-> global block st*8 + rank.
    out_d = nc.dram_tensor("out", [4, P, E], bf16, kind="ExternalOutput")
    warm_in = nc.dram_tensor("warm_in", [NCORES, 1, 16], bf16)
    warm_out = nc.dram_tensor("warm_out", [NCORES, 1, 16], bf16)
    a2a_in = [nc.dram_tensor(f"a2a_in{b}", [NCORES, P, 2 * P], bf16)
              for b in range(B)]
    a2a_out = [nc.dram_tensor(f"a2a_out{b}", [NCORES, P, 2 * P], bf16)
               for b in range(B)]

    with tile.TileContext(nc) as tc, ExitStack() as ctx:
        consts = ctx.enter_context(tc.tile_pool(name="consts", bufs=1))
        work = ctx.enter_context(tc.tile_pool(name="work", bufs=1))
        xpool = ctx.enter_context(tc.tile_pool(name="xstream", bufs=2))
        epool = ctx.enter_context(tc.tile_pool(name="expst", bufs=2))
        small = ctx.enter_context(tc.tile_pool(name="small", bufs=2))
        opool = ctx.enter_context(tc.tile_pool(name="osb", bufs=1))
        pbig = ctx.enter_context(tc.tile_pool(name="pbig", bufs=2, space="PSUM"))
        ppv = ctx.enter_context(tc.tile_pool(name="ppv", bufs=1, space="PSUM"))
        psm = ctx.enter_context(tc.tile_pool(name="psm", bufs=2, space="PSUM"))

        wqk = consts.tile([P, EB, 2 * P], bf16, tag="wqk")
        wv = consts.tile([P, EB, P], bf16, tag="wv")
        bqk = consts.tile([P, 2], f32, tag="bqk")
        bo = consts.tile([P, E], bf16, tag="bo")
        tri = consts.tile([P, P], bf16, tag="tri")
        ones1 = consts.tile([1, P], bf16, tag="ones1")
        # all-ones column block: row 64 serves as the K=1 stationary
        # operand that broadcasts the denominator row (also at partition 64)
        onesc = consts.tile([P, D], bf16, tag="onesc")

        nc.vector.memset(ones1[:1, :], 1.0)
        nc.vector.memset(onesc[:], 1.0)

        qkT = [work.tile([P, 2, S], bf16, tag=f"qkT{b}", name=f"qkT{b}")
               for b in range(B)]
        vsb = [work.tile([P, SBB, HPC, 66], bf16, tag=f"vsb{b}", name=f"vsb{b}")
               for b in range(B)]
        # per-head attnT halves (both on partitions 0-63): keeps every DVE
        # normalize op partition-aligned; the bounce DMA does the shift of
        # head 1 into partitions 64-127 of the A2A payload
        attnT = [[work.tile([D, S], bf16, tag=f"attnT{h}",
                            name=f"attnT{b}{h}") for h in range(HPC)]
                 for b in range(B)]

        def qkv_pieces(b):
            """QKV projection for batch b, one 512-token chunk per piece.

            Chunks are emitted suffix-first: causal score block kb only needs
            token columns >= kb*128, so late chunks unblock the small k-blocks
            early and ACT (exp) can start before the whole projection is done.
            """
            nc.vector.memset(vsb[b][:], 1.0)
            for i, sc in enumerate(reversed(range(S // 512))):
                gc = b * S + sc * 512  # global col
                xc = xpool.tile([P, EB, 512], bf16, tag="xc", name="xc")
                if b == 0 and i == 0:
                    # interleave wqk/x per-eb so matmul eb=0 can start
                    # after ~1/8 of the startup bytes; remaining consts
                    # queue behind it off the critical path
                    for eb in range(EB):
                        nc.sync.dma_start(wqk[:, eb, :], wqk_d[:, eb, :])
                        nc.sync.dma_start(xc[:, eb, :],
                                          xT_d[:, eb, gc:gc + 512])
                        if eb == 0:
                            nc.sync.dma_start(bqk[:], bqk_d[:, :])
                    nc.sync.dma_start(wv[:], wv_d[:, :, :])
                    nc.sync.dma_start(tri[:], tri_d[:, :])
                else:
                    nc.sync.dma_start(xc[:], xT_d[:, :, gc:gc + 512])
                for db in range(2):
                    ps = psm.tile([P, 512], f32, tag="mid", name="psqk")
                    for eb in range(EB):
                        nc.tensor.matmul(
                            ps[:],
                            lhsT=wqk[:, eb, db * P:(db + 1) * P],
                            rhs=xc[:, eb, :],
                            start=(eb == 0), stop=(eb == EB - 1),
                        )
                    nc.vector.tensor_scalar_add(
                        qkT[b][:, db, sc * 512:(sc + 1) * 512], ps[:],
                        bqk[:, db:db + 1])
                    yield
                for si in range(4):
                    sb = sc * 4 + si
                    pv_ = psm.tile([P, P], f32, tag="mid", name="psv")
                    for eb in range(EB):
                        nc.tensor.matmul(
                            pv_[:], lhsT=xc[:, eb, si * P:(si + 1) * P],
                            rhs=wv[:, eb, :], start=(eb == 0),
                            stop=(eb == EB - 1))
                    # v bias is NOT added here: softmax rows sum to 1, so
                    # bv@W_o folds into b_o host-side (exact); one 3D-AP
                    # copy drops both heads' slices in place
                    nc.vector.tensor_copy(
                        vsb[b][:, sb, :, 0:64],
                        pv_[:].rearrange("p (h d) -> p h d", h=2))
                    yield

        def score_pieces(b, h, expst, order=None):
            """scores^T + exp for one (batch, head), one k-block per piece.

            Default k-block order is high-to-low, matching qkv_pieces'
            suffix-first chunks. Batch 1 uses middle-out ([15..8, 0..7]) so
            its PV q-tiles unlock incrementally as the low k-blocks arrive.
            """
            hs = slice(h * 64, (h + 1) * 64)
            if not expst:
                expst.extend([None] * SBB)
            if order is None:
                order = list(reversed(range(SBB)))
            for kb in order:
                L = S - kb * P
                # 4 bufs: both batches' tiles live concurrently, so
                # batch-1 exps never wait on batch-0's PV to release slots
                et = epool.tile([P, L], bf16, tag=f"e{kb}", name=f"e{kb}",
                                bufs=4)
                off = kb * P
                pos = 0
                while pos < L:  # 1024-wide psum tiles: 1 exp op per tile
                    c = min(1024, L - pos)
                    ps = pbig.tile([P, 1024], f32, tag="big", name="pssc")
                    for c0 in range(0, c, 512):
                        w = min(512, c - c0)
                        nc.tensor.matmul(
                            ps[:, c0:c0 + w],
                            lhsT=qkT[b][hs, 1, off:off + P],
                            rhs=qkT[b][hs, 0, off + pos + c0:off + pos + c0 + w],
                            start=True, stop=True)
                    nc.scalar.activation(
                        et[:, pos:pos + c], ps[:, :c],
                        mybir.ActivationFunctionType.Exp)
                    pos += c
                # zero the invalid (q < k) half of the diagonal block.
                # DVE (not GpSimd): keeps the gpsimd queue empty so the
                # collective triggers fire as soon as their DMAs land.
                nc.vector.tensor_mul(et[:, 0:P], et[:, 0:P], tri[:])
                expst[kb] = et
                yield

        def pv_pieces(b, e0, e1):
            """Flipped PV for batch b: one (q-tile, head) chain per piece.

            out[d, q] = sum_kb vsb[kb]^T @ expst[kb][:, qwin]: N=512 moving
            columns per matmul, stationary operand only 65 columns, so the
            PE stays matmul-bound (no LDWEIGHTS stalls, no HAM cooldown).
            Row 64 accumulates the softmax denominator (ones column of vsb).
            After both heads' chains for a q-tile: reciprocal rows ->
            K=2 broadcast matmul -> two DVE mults write attnT normalized.
            """
            expst = (e0, e1)
            for qt in range(NQT):
                q0 = qt * QT
                pvs = [None, None]
                for h in range(HPC):
                    pp = ppv.tile([65, QT], f32, tag=f"pv{h}",
                                  name=f"pv{h}")
                    nkb = 4 * qt + 4  # k-blocks touching this q-tile
                    for kb in range(nkb):
                        ecol = q0 - kb * P  # expst col of q-tile start
                        poff = max(0, -ecol)
                        w = QT - poff
                        nc.tensor.matmul(
                            pp[:, poff:QT],
                            lhsT=vsb[b][:, kb, h, 0:65],
                            rhs=expst[h][kb][:, ecol + poff:ecol + poff + w],
                            start=(kb == 0), stop=(kb == nkb - 1))
                    # fast-release: one DVE copy frees the PSUM slot so the
                    # next chain never waits on the normalize tail
                    pvs[h] = small.tile([65, QT], bf16, tag=f"pvs{h}",
                                        name=f"pvs{h}", bufs=1)
                    nc.vector.tensor_copy(pvs[h][:], pp[:, :])
                    yield
                # broadcast each raw denominator row across 64 partitions
                # with a K=1 matmul, take the reciprocal on the broadcast
                # (per-lane cost is free-dim-bound, so this costs the same
                # as a single-row reciprocal but needs no extra copy), then
                # normalize into the head's attnT half
                for h in range(HPC):
                    bc = psm.tile([D, QT], f32, tag="mid", name="bc")
                    nc.tensor.matmul(bc[0:D, :],
                                     lhsT=onesc[64:65, 0:D],
                                     rhs=pvs[h][64:65, :],
                                     start=True, stop=True)
                    bcs = small.tile([D, QT], f32, tag="bcs",
                                     name=f"bcs{h}", bufs=1)
                    nc.vector.reciprocal_approx_fast(out=bcs[:],
                                                     in_=bc[0:D, :])
                    nc.vector.tensor_mul(attnT[b][h][0:D, q0:q0 + QT],
                                         pvs[h][0:D, :], bcs[0:D, :])
                yield

        def interleave(*gens):
            gens = list(gens)
            while gens:
                gens = [g for g in gens if next(g, StopIteration) is not StopIteration]

        def paced(qg, score_gens, pv_gens=(), pv_every=1):
            """Weave one qkv stream with score/pv streams, pacing emission so
            every score k-block is emitted AFTER the qkv chunk that writes the
            qkT columns it reads (Tile only tracks writer->reader deps in
            emission order). qkv chunk g (suffix-first) unlocks score k-blocks
            [12-4g, 15-4g]."""
            rnd = 0
            for g in range(4):
                for _ in range(6):
                    next(qg, None)
                for _ in range(4):
                    for sg in score_gens:
                        next(sg, None)
                    if rnd % pv_every == 0:
                        for pg in pv_gens:
                            next(pg, None)
                    rnd += 1
            interleave(qg, *score_gens, *pv_gens)

        atf = [work.tile([P, EB, 2 * P], bf16, tag="atf",
                         name=f"atf{b}") for b in range(B)]

        def bounce(b):
            """attnT -> a2a_in: chunk j of the bounce gets token blocks
            {j, j+8}; head h's 64 rows land at payload partitions h*64+.
            On the otherwise-empty GpSimd SWDGE queue so the collective
            trigger right behind it fires immediately."""
            for t in range(2):
                for h in range(HPC):
                    nc.gpsimd.dma_start(
                        a2a_in[b].ap().rearrange(
                            "j p (t c) -> p j t c",
                            t=2)[h * D:(h + 1) * D, :, t, :],
                        attnT[b][h][:, t * NCORES * P:(t + 1) * NCORES * P]
                        .rearrange("p (j c) -> p j c", c=P))

        def a2a_batch(b):
            """AllToAll of batch b (512KB per rank)."""
            if no_cc:
                for j in range(NCORES):
                    nc.sync.dma_start(a2a_out[b][j], a2a_in[b][j])
            else:
                nc.gpsimd.collective_compute(
                    "AllToAll", mybir.AluOpType.bypass,
                    replica_groups=[list(range(NCORES))],
                    ins=[a2a_in[b].ap()], outs=[a2a_out[b].ap()])

        def atf_gather(b):
            nc.sync.dma_start(
                atf[b][:, :, :],
                a2a_out[b].ap().rearrange("j p c -> p j c"))

        def oproj_half(b, st):
            """Output projection of token block st*8 + rank of batch b.
            b_o is host-broadcast to all partitions, so the bias rides the
            PSUM->SBUF copyout as a DVE add (no K=1 bias matmuls)."""
            ot = opool.tile([P, E], bf16, tag="o", name="ot")
            po = pbig.tile([P, 1024], f32, tag="big", name="pso")
            for oh in range(2):
                for eb in range(EB):
                    nc.tensor.matmul(
                        po[:, oh * 512:(oh + 1) * 512],
                        lhsT=atf[b][:, eb, st * P:(st + 1) * P],
                        rhs=woh[oh][:, eb, :],
                        start=(eb == 0), stop=(eb == EB - 1))
            nc.vector.tensor_tensor(out=ot[:], in0=po[:], in1=bo[:],
                                    op=mybir.AluOpType.add)
            nc.sync.dma_start(out_d[b * 2 + st], ot[:])

        # ---- pipelined emission (priorities; Tile schedules by readiness) ----
        # Phase-sequential PE stream (in-order engine queues make fine
        # interleaving counterproductive): batch-0 QKV+scores, batch-1
        # QKV+scores (ACT exps trail), then both PV phases back to back --
        # each triggers its half-AllToAlls as attnT halves complete -- and
        # the four output projections last, overlapping the tail collectives.
        # fire a tiny AllToAll immediately (gpsimd queue is empty, no
        # deps): it runs right after the framework barrier and warms the
        # CC data path ~70us before the first real collective, which
        # otherwise runs at less than half the second one's bandwidth
        if not no_cc:
            nc.gpsimd.collective_compute(
                "AllToAll", mybir.AluOpType.bypass,
                replica_groups=[list(range(NCORES))],
                ins=[warm_in.ap()], outs=[warm_out.ap()])
        e00, e01, e10, e11 = [], [], [], []
        paced(qkv_pieces(0),
              [score_pieces(0, 0, e00), score_pieces(0, 1, e01)])
        nc.sync.dma_start(bo[:, :], bo_d[:, :])
        # batch 1 middle-out: high k-blocks pace with the suffix-first qkv
        # chunks; the low half is emitted only after ALL qkv1 pieces (its
        # matmuls read every qkT column -- emission order must respect
        # writer->reader) and runs forward so pv(1) unlocks incrementally.
        # k-blocks 7..4 need only qkv1 chunks sc>=1 (stationary k-cols
        # 512-1023, moving q-cols >= 512), so they pace inside phase 2's
        # rounds; only kb 0-3 must trail the final chunk -- this pulls
        # ~10us of exp off the post-projection critical path
        mid_hi = list(reversed(range(8, SBB))) + [7, 6, 5, 4]
        mid_lo = list(range(4))
        paced(qkv_pieces(1),
              [score_pieces(1, 0, e10, mid_hi),
               score_pieces(1, 1, e11, mid_hi)])
        # W_o halves land in the two xc slots the moment QKV stops using
        # them (same shape/tag); loaded well before the first oproj
        woh = [xpool.tile([P, EB, 512], bf16, tag="xc", name=f"wo{oh}")
               for oh in range(2)]
        for oh in range(2):
            nc.sync.dma_start(woh[oh][:], wo_d[:, :, oh * 512:(oh + 1) * 512])
        # PV(0) woven WITH batch-1's low score blocks: pv0's exps are long
        # done so its chains fill the PE while scores1lo is ACT-bound, and
        # finishing pv0 here fires A2A(0) ~18us earlier so oproj(0) never
        # waits on it at the tail; scores1lo's exp schedule (and so pv1)
        # is unchanged.
        s1lo = [score_pieces(1, 0, e10, mid_lo),
                score_pieces(1, 1, e11, mid_lo)]
        p0 = pv_pieces(0, e00, e01)
        for _ in range(8):
            for sg in s1lo:
                next(sg, None)
            for _ in range(3):
                next(p0, None)
        interleave(p0, *s1lo)
        bounce(0)
        a2a_batch(0)            # overlaps batch-1 PV
        interleave(pv_pieces(1, e10, e11))
        bounce(1)
        a2a_batch(1)            # overlaps oproj of batch 0
        atf_gather(0)
        atf_gather(1)
        oproj_half(0, 0)
        oproj_half(0, 1)
        oproj_half(1, 0)
        oproj_half(1, 1)

    nc.compile()
    return nc


def _in_maps(x, W_qkv, b_qkv, W_o, b_o):
    # [partition, eblock, col] layouts (see dram tensor decls)
    xT = np.ascontiguousarray(
        x.reshape(BS, EB, P).transpose(2, 1, 0)).astype(_bf16)
    wo = np.ascontiguousarray(
        W_o.reshape(EB, P, E).transpose(1, 0, 2)).astype(_bf16)
    # fold the v bias through the output projection: softmax rows sum to
    # 1, so attn = softmax@v + bv and out = softmax@v@W_o + (bv@W_o + b_o)
    bo2 = np.asarray(b_o, np.float64) + np.asarray(
        b_qkv[2 * E:], np.float64) @ np.asarray(W_o, np.float64)
    bo = np.ascontiguousarray(np.broadcast_to(
        bo2.reshape(1, E), (P, E))).astype(_bf16)
    tri = np.triu(np.ones((P, P), np.float32)).astype(_bf16)
    maps = []
    for c in range(NCORES):
        o = c * HPC * D
        q_sl = slice(o, o + HPC * D)
        k_sl = slice(E + o, E + o + HPC * D)
        v_sl = slice(2 * E + o, 2 * E + o + HPC * D)
        wqk = np.concatenate(
            [W_qkv[:, q_sl] * 0.125, W_qkv[:, k_sl]], axis=1)
        maps.append({
            "xT": xT,
            "wqk": np.ascontiguousarray(
                wqk.reshape(EB, P, 2 * P).transpose(1, 0, 2)).astype(_bf16),
            "wv": np.ascontiguousarray(
                W_qkv[:, v_sl].reshape(EB, P, P).transpose(1, 0, 2)).astype(_bf16),
            "wo": wo,
            "bqk": np.stack([b_qkv[q_sl] * 0.125,
                             b_qkv[k_sl]], axis=1).astype(np.float32),
            "bo": bo,
            "tri": tri,
        })
    return maps


def kernel(x, W_qkv, b_qkv, W_o, b_o, mask):
    from concourse.bass_utils import run_bass_kernel_spmd

    if "nc" not in _cache:
        _cache["nc"] = _build()
    nc = _cache["nc"]
    maps = _in_maps(np.asarray(x, np.float32), np.asarray(W_qkv, np.float32),
                    np.asarray(b_qkv, np.float32), np.asarray(W_o, np.float32),
                    np.asarray(b_o, np.float32))
    res = run_bass_kernel_spmd(nc, maps, list(range(NCORES)))
    # rank r's out[st] is global 128-token block st*8 + r
    full = np.empty((SB, P, E), np.float32)
    for r in range(NCORES):
        full[r::NCORES] = res.results[r]["out"]
    return full.reshape(B, S, E).astype(np.float32)



# revision 31
# speedup vs baseline: 1.1282x; 1.0790x over previous
"""Multi-head causal self-attention (B=2, S=2048, E=1024, H=16) on 8 TRN2 NeuronCores.

Sharding: tensor-parallel over heads (2 heads/core, both batches). Per core:
  - QKV projection for its 2 heads (q^T/k^T transposed layout, v natural;
    both biases are folded away: softmax rows sum to 1, so bv rides
    through the output projection as bo' = bo + bv@W_o, computed host-side)
  - causal flash-style attention, scores computed transposed (k on
    partitions) and exp'd on ACT in 1024-wide PSUM tiles; tri-mask on DVE
  - PV computed FLIPPED (lhsT = [v | ones], rhs = exp(scores^T)): each
    matmul streams 512 q-columns (matmul-bound, no LDWEIGHTS stalls) and
    the result lands pre-transposed [d, q] with the softmax denominator in
    row 64; a K=1 matmul broadcasts the denominator row across 64
    partitions, reciprocal_approx_fast inverts the broadcast, one DVE
    multiply normalizes into per-head attnT halves (all partition-aligned)
  - one 512KB AllToAll per batch, triggered from the otherwise-empty
    GpSimd queue the moment that batch's attnT completes (collectives
    have a ~13us fixed cost, so fewer, larger A2As beat four half-sized
    ones), then four output projections (bias added by the DVE copyout
    against a host-broadcast bo'), each overlapping the next collective.
Emission is phase-sequential (matching HAM clock-gate behavior: dense PE
phases stay at 8/8): qkv0+scores0 (reversed, paced), qkv1+scores1
(middle-out: high k-blocks paced with the suffix-first chunks), then the
batch-1 low score blocks INTERLEAVED with pv0 (pv0's exps are long done,
so its chains fill the PE while scores1lo is ACT-bound, and A2A(0) fires
~18us earlier -- early enough that oproj(0) never waits on it), pv1,
A2A(1), oproj x4 (bf16 out DMAs). Startup interleaves wqk/x chunk-0 loads
per e-block so the first matmul starts after ~1/8 of the startup bytes.
All 16 exp tiles per (batch, head) keep 4 slot buffers so batch-1 exps
never wait on batch-0's PV to release them. Host side only reshapes/
slices inputs and concatenates the 8 disjoint row shards of the output.
"""

import numpy as np
import ml_dtypes

P = 128
B, S, E, H, D = 2, 2048, 1024, 16, 64
NCORES = 8
EB = E // P            # 8 e-blocks
BS = B * S             # 4096 flattened rows
SBB = S // P           # 16 s-blocks per batch
SB = BS // P           # 32 s-blocks global
HPC = H // NCORES      # 2 heads per core
CH = BS // NCORES      # 512 rows owned per core
QT = 512               # q-tile width for the PV phase
NQT = S // QT          # 4 q-tiles per batch

_bf16 = ml_dtypes.bfloat16
_cache = {}


def _build(no_cc=False):
    from contextlib import ExitStack

    import concourse.tile as tile
    from concourse import bacc, mybir

    bf16 = mybir.dt.bfloat16
    f32 = mybir.dt.float32

    nc = bacc.Bacc("TRN2", target_bir_lowering=False, debug=False,
                   num_devices=NCORES)

    # host-side layouts are [partition, eblock, col] so each load is one DMA
    xT_d = nc.dram_tensor("xT", [P, EB, BS], bf16, kind="ExternalInput")
    wqk_d = nc.dram_tensor("wqk", [P, EB, 2 * P], bf16, kind="ExternalInput")
    wv_d = nc.dram_tensor("wv", [P, EB, P], bf16, kind="ExternalInput")
    wo_d = nc.dram_tensor("wo", [P, EB, E], bf16, kind="ExternalInput")
    bqk_d = nc.dram_tensor("bqk", [P, 2], f32, kind="ExternalInput")
    bo_d = nc.dram_tensor("bo", [P, E], bf16, kind="ExternalInput")
    tri_d = nc.dram_tensor("tri", [P, P], bf16, kind="ExternalInput")
    # rank r owns interleaved token blocks {r, r+8, r+16, r+24}: one AllToAll
    # per batch. out row-block st <-> global block st*8 + rank.
    out_d = nc.dram_tensor("out", [4, P, E], bf16, kind="ExternalOutput")
    warm_in = nc.dram_tensor("warm_in", [NCORES, 1, 16], bf16)
    warm_out = nc.dram_tensor("warm_out", [NCORES, 1, 16], bf16)
    a2a_in = [nc.dram_tensor(f"a2a_in{b}", [NCORES, P, 2 * P], bf16)
              for b in range(B)]
    a2a_out = [nc.dram_tensor(f"a2a_out{b}", [NCORES, P, 2 * P], bf16)
               for b in range(B)]

    with tile.TileContext(nc) as tc, ExitStack() as ctx:
        consts = ctx.enter_context(tc.tile_pool(name="consts", bufs=1))
        work = ctx.enter_context(tc.tile_pool(name="work", bufs=1))
        xpool = ctx.enter_context(tc.tile_pool(name="xstream", bufs=2))
        epool = ctx.enter_context(tc.tile_pool(name="expst", bufs=2))
        small = ctx.enter_context(tc.tile_pool(name="small", bufs=2))
        opool = ctx.enter_context(tc.tile_pool(name="osb", bufs=1))
        pbig = ctx.enter_context(tc.tile_pool(name="pbig", bufs=2, space="PSUM"))
        ppv = ctx.enter_context(tc.tile_pool(name="ppv", bufs=1, space="PSUM"))
        psm = ctx.enter_context(tc.tile_pool(name="psm", bufs=2, space="PSUM"))

        wqk = consts.tile([P, EB, 2 * P], bf16, tag="wqk")
        wv = consts.tile([P, EB, P], bf16, tag="wv")
        bqk = consts.tile([P, 2], f32, tag="bqk")
        bo = consts.tile([P, E], bf16, tag="bo")
        tri = consts.tile([P, P], bf16, tag="tri")
        ones1 = consts.tile([1, P], bf16, tag="ones1")
        # all-ones column block: row 64 serves as the K=1 stationary
        # operand that broadcasts the denominator row (also at partition 64)
        onesc = consts.tile([P, D], bf16, tag="onesc")

        nc.vector.memset(ones1[:1, :], 1.0)
        nc.vector.memset(onesc[:], 1.0)

        qkT = [work.tile([P, 2, S], bf16, tag=f"qkT{b}", name=f"qkT{b}")
               for b in range(B)]
        vsb = [work.tile([P, SBB, HPC, 66], bf16, tag=f"vsb{b}", name=f"vsb{b}")
               for b in range(B)]
        # per-head attnT halves (both on partitions 0-63): keeps every DVE
        # normalize op partition-aligned; the bounce DMA does the shift of
        # head 1 into partitions 64-127 of the A2A payload
        attnT = [[work.tile([D, S], bf16, tag=f"attnT{h}",
                            name=f"attnT{b}{h}") for h in range(HPC)]
                 for b in range(B)]

        def qkv_pieces(b):
            """QKV projection for batch b, one 512-token chunk per piece.

            Chunks are emitted suffix-first: causal score block kb only needs
            token columns >= kb*128, so late chunks unblock the small k-blocks
            early and ACT (exp) can start before the whole projection is done.
            """
            nc.vector.memset(vsb[b][:], 1.0)
            for i, sc in enumerate(reversed(range(S // 512))):
                gc = b * S + sc * 512  # global col
                xc = xpool.tile([P, EB, 512], bf16, tag="xc", name="xc")
                if b == 0 and i == 0:
                    # interleave wqk/x per-eb so matmul eb=0 can start
                    # after ~1/8 of the startup bytes; remaining consts
                    # queue behind it off the critical path
                    for eb in range(EB):
                        nc.sync.dma_start(wqk[:, eb, :], wqk_d[:, eb, :])
                        nc.sync.dma_start(xc[:, eb, :],
                                          xT_d[:, eb, gc:gc + 512])
                        if eb == 0:
                            nc.sync.dma_start(bqk[:], bqk_d[:, :])
                    nc.sync.dma_start(wv[:], wv_d[:, :, :])
                    nc.sync.dma_start(tri[:], tri_d[:, :])
                else:
                    nc.sync.dma_start(xc[:], xT_d[:, :, gc:gc + 512])
                for db in range(2):
                    ps = psm.tile([P, 512], f32, tag="mid", name="psqk")
                    for eb in range(EB):
                        nc.tensor.matmul(
                            ps[:],
                            lhsT=wqk[:, eb, db * P:(db + 1) * P],
                            rhs=xc[:, eb, :],
                            start=(eb == 0), stop=(eb == EB - 1),
                        )
                    nc.vector.tensor_scalar_add(
                        qkT[b][:, db, sc * 512:(sc + 1) * 512], ps[:],
                        bqk[:, db:db + 1])
                    yield
                for si in range(4):
                    sb = sc * 4 + si
                    pv_ = psm.tile([P, P], f32, tag="mid", name="psv")
                    for eb in range(EB):
                        nc.tensor.matmul(
                            pv_[:], lhsT=xc[:, eb, si * P:(si + 1) * P],
                            rhs=wv[:, eb, :], start=(eb == 0),
                            stop=(eb == EB - 1))
                    # v bias is NOT added here: softmax rows sum to 1, so
                    # bv@W_o folds into b_o host-side (exact); one 3D-AP
                    # copy drops both heads' slices in place
                    nc.vector.tensor_copy(
                        vsb[b][:, sb, :, 0:64],
                        pv_[:].rearrange("p (h d) -> p h d", h=2))
                    yield

        def score_pieces(b, h, expst, order=None):
            """scores^T + exp for one (batch, head), one k-block per piece.

            Default k-block order is high-to-low, matching qkv_pieces'
            suffix-first chunks. Batch 1 uses middle-out ([15..8, 0..7]) so
            its PV q-tiles unlock incrementally as the low k-blocks arrive.
            """
            hs = slice(h * 64, (h + 1) * 64)
            if not expst:
                expst.extend([None] * SBB)
            if order is None:
                order = list(reversed(range(SBB)))
            for kb in order:
                L = S - kb * P
                # 4 bufs: both batches' tiles live concurrently, so
                # batch-1 exps never wait on batch-0's PV to release slots
                et = epool.tile([P, L], bf16, tag=f"e{kb}", name=f"e{kb}",
                                bufs=4)
                off = kb * P
                pos = 0
                while pos < L:  # 1024-wide psum tiles: 1 exp op per tile
                    c = min(1024, L - pos)
                    ps = pbig.tile([P, 1024], f32, tag="big", name="pssc")
                    for c0 in range(0, c, 512):
                        w = min(512, c - c0)
                        nc.tensor.matmul(
                            ps[:, c0:c0 + w],
                            lhsT=qkT[b][hs, 1, off:off + P],
                            rhs=qkT[b][hs, 0, off + pos + c0:off + pos + c0 + w],
                            start=True, stop=True)
                    nc.scalar.activation(
                        et[:, pos:pos + c], ps[:, :c],
                        mybir.ActivationFunctionType.Exp)
                    pos += c
                # zero the invalid (q < k) half of the diagonal block.
                # DVE (not GpSimd): keeps the gpsimd queue empty so the
                # collective triggers fire as soon as their DMAs land.
                nc.vector.tensor_mul(et[:, 0:P], et[:, 0:P], tri[:])
                expst[kb] = et
                yield

        def pv_pieces(b, e0, e1):
            """Flipped PV for batch b: one (q-tile, head) chain per piece.

            out[d, q] = sum_kb vsb[kb]^T @ expst[kb][:, qwin]: N=512 moving
            columns per matmul, stationary operand only 65 columns, so the
            PE stays matmul-bound (no LDWEIGHTS stalls, no HAM cooldown).
            Row 64 accumulates the softmax denominator (ones column of vsb).
            After both heads' chains for a q-tile: reciprocal rows ->
            K=2 broadcast matmul -> two DVE mults write attnT normalized.
            """
            expst = (e0, e1)
            for qt in range(NQT):
                q0 = qt * QT
                pvs = [None, None]
                for h in range(HPC):
                    pp = ppv.tile([65, QT], f32, tag=f"pv{h}",
                                  name=f"pv{h}")
                    nkb = 4 * qt + 4  # k-blocks touching this q-tile
                    for kb in range(nkb):
                        ecol = q0 - kb * P  # expst col of q-tile start
                        poff = max(0, -ecol)
                        w = QT - poff
                        nc.tensor.matmul(
                            pp[:, poff:QT],
                            lhsT=vsb[b][:, kb, h, 0:65],
                            rhs=expst[h][kb][:, ecol + poff:ecol + poff + w],
                            start=(kb == 0), stop=(kb == nkb - 1))
                    # fast-release: one DVE copy frees the PSUM slot so the
                    # next chain never waits on the normalize tail
                    pvs[h] = small.tile([65, QT], bf16, tag=f"pvs{h}",
                                        name=f"pvs{h}", bufs=1)
                    nc.vector.tensor_copy(pvs[h][:], pp[:, :])
                    yield
                # broadcast each raw denominator row across 64 partitions
                # with a K=1 matmul, take the reciprocal on the broadcast
                # (per-lane cost is free-dim-bound, so this costs the same
                # as a single-row reciprocal but needs no extra copy), then
                # normalize into the head's attnT half
                for h in range(HPC):
                    bc = psm.tile([D, QT], f32, tag="mid", name="bc")
                    nc.tensor.matmul(bc[0:D, :],
                                     lhsT=onesc[64:65, 0:D],
                                     rhs=pvs[h][64:65, :],
                                     start=True, stop=True)
                    bcs = small.tile([D, QT], f32, tag="bcs",
                                     name=f"bcs{h}", bufs=1)
                    nc.vector.reciprocal_approx_fast(out=bcs[:],
                                                     in_=bc[0:D, :])
                    nc.vector.tensor_mul(attnT[b][h][0:D, q0:q0 + QT],
                                         pvs[h][0:D, :], bcs[0:D, :])
                yield

        def interleave(*gens):
            gens = list(gens)
            while gens:
                gens = [g for g in gens if next(g, StopIteration) is not StopIteration]

        def paced(qg, score_gens, pv_gens=(), pv_every=1):
            """Weave one qkv stream with score/pv streams, pacing emission so
            every score k-block is emitted AFTER the qkv chunk that writes the
            qkT columns it reads (Tile only tracks writer->reader deps in
            emission order). qkv chunk g (suffix-first) unlocks score k-blocks
            [12-4g, 15-4g]."""
            rnd = 0
            for g in range(4):
                for _ in range(6):
                    next(qg, None)
                for _ in range(4):
                    for sg in score_gens:
                        next(sg, None)
                    if rnd % pv_every == 0:
                        for pg in pv_gens:
                            next(pg, None)
                    rnd += 1
            interleave(qg, *score_gens, *pv_gens)

        atf = [work.tile([P, EB, 2 * P], bf16, tag="atf",
                         name=f"atf{b}") for b in range(B)]

        def bounce(b):
            """attnT -> a2a_in: chunk j of the bounce gets token blocks
            {j, j+8}; head h's 64 rows land at payload partitions h*64+.
            On the otherwise-empty GpSimd SWDGE queue so the collective
            trigger right behind it fires immediately."""
            for t in range(2):
                for h in range(HPC):
                    nc.gpsimd.dma_start(
                        a2a_in[b].ap().rearrange(
                            "j p (t c) -> p j t c",
                            t=2)[h * D:(h + 1) * D, :, t, :],
                        attnT[b][h][:, t * NCORES * P:(t + 1) * NCORES * P]
                        .rearrange("p (j c) -> p j c", c=P))

        def a2a_batch(b):
            """AllToAll of batch b (512KB per rank)."""
            if no_cc:
                for j in range(NCORES):
                    nc.sync.dma_start(a2a_out[b][j], a2a_in[b][j])
            else:
                nc.gpsimd.collective_compute(
                    "AllToAll", mybir.AluOpType.bypass,
                    replica_groups=[list(range(NCORES))],
                    ins=[a2a_in[b].ap()], outs=[a2a_out[b].ap()])

        def atf_gather(b):
            nc.sync.dma_start(
                atf[b][:, :, :],
                a2a_out[b].ap().rearrange("j p c -> p j c"))

        def oproj_half(b, st):
            """Output projection of token block st*8 + rank of batch b.
            b_o is host-broadcast to all partitions, so the bias rides the
            PSUM->SBUF copyout as a DVE add (no K=1 bias matmuls)."""
            ot = opool.tile([P, E], bf16, tag="o", name="ot")
            po = pbig.tile([P, 1024], f32, tag="big", name="pso")
            for oh in range(2):
                for eb in range(EB):
                    nc.tensor.matmul(
                        po[:, oh * 512:(oh + 1) * 512],
                        lhsT=atf[b][:, eb, st * P:(st + 1) * P],
                        rhs=woh[oh][:, eb, :],
                        start=(eb == 0), stop=(eb == EB - 1))
            nc.vector.tensor_tensor(out=ot[:], in0=po[:], in1=bo[:],
                                    op=mybir.AluOpType.add)
            nc.sync.dma_start(out_d[b * 2 + st], ot[:])

        # ---- pipelined emission (priorities; Tile schedules by readiness) ----
        # Phase-sequential PE stream (in-order engine queues make fine
        # interleaving counterproductive): batch-0 QKV+scores, batch-1
        # QKV+scores (ACT exps trail), then both PV phases back to back --
        # each triggers its half-AllToAlls as attnT halves complete -- and
        # the four output projections last, overlapping the tail collectives.
        # tiny AllToAll fired at kernel start (gpsimd queue empty, no
        # deps): runs right after the framework barrier and absorbs the
        # first-collective rank-alignment cost -- measured to cut A2A(0)'s
        # start skew from ~11.5us to ~1.1us
        if not no_cc:
            nc.gpsimd.collective_compute(
                "AllToAll", mybir.AluOpType.bypass,
                replica_groups=[list(range(NCORES))],
                ins=[warm_in.ap()], outs=[warm_out.ap()])
        e00, e01, e10, e11 = [], [], [], []
        paced(qkv_pieces(0),
              [score_pieces(0, 0, e00), score_pieces(0, 1, e01)])
        nc.sync.dma_start(bo[:, :], bo_d[:, :])
        # batch 1 middle-out: high k-blocks pace with the suffix-first qkv
        # chunks; the low half is emitted only after ALL qkv1 pieces (its
        # matmuls read every qkT column -- emission order must respect
        # writer->reader) and runs forward so pv(1) unlocks incrementally.
        # k-blocks 7..4 need only qkv1 chunks sc>=1 (stationary k-cols
        # 512-1023, moving q-cols >= 512), so they pace inside phase 2's
        # rounds; only kb 0-3 must trail the final chunk -- this pulls
        # ~10us of exp off the post-projection critical path
        mid_hi = list(reversed(range(8, SBB))) + [7, 6, 5, 4]
        mid_lo = list(range(4))
        paced(qkv_pieces(1),
              [score_pieces(1, 0, e10, mid_hi),
               score_pieces(1, 1, e11, mid_hi)])
        # W_o halves land in the two xc slots the moment QKV stops using
        # them (same shape/tag); loaded well before the first oproj
        woh = [xpool.tile([P, EB, 512], bf16, tag="xc", name=f"wo{oh}")
               for oh in range(2)]
        for oh in range(2):
            nc.sync.dma_start(woh[oh][:], wo_d[:, :, oh * 512:(oh + 1) * 512])
        # PV(0) woven WITH batch-1's low score blocks: pv0's exps are long
        # done so its chains fill the PE while scores1lo is ACT-bound, and
        # finishing pv0 here fires A2A(0) ~18us earlier so oproj(0) never
        # waits on it at the tail; scores1lo's exp schedule (and so pv1)
        # is unchanged.
        s1lo = [score_pieces(1, 0, e10, mid_lo),
                score_pieces(1, 1, e11, mid_lo)]
        p0 = pv_pieces(0, e00, e01)
        for _ in range(8):
            for sg in s1lo:
                next(sg, None)
            next(p0, None)
            next(p0, None)
        interleave(p0, *s1lo)
        bounce(0)
        a2a_batch(0)            # overlaps batch-1 PV
        interleave(pv_pieces(1, e10, e11))
        bounce(1)
        a2a_batch(1)            # overlaps oproj of batch 0
        atf_gather(0)
        atf_gather(1)
        oproj_half(0, 0)
        oproj_half(0, 1)
        oproj_half(1, 0)
        oproj_half(1, 1)

    nc.compile()
    return nc


def _in_maps(x, W_qkv, b_qkv, W_o, b_o):
    # [partition, eblock, col] layouts (see dram tensor decls)
    xT = np.ascontiguousarray(
        x.reshape(BS, EB, P).transpose(2, 1, 0)).astype(_bf16)
    wo = np.ascontiguousarray(
        W_o.reshape(EB, P, E).transpose(1, 0, 2)).astype(_bf16)
    # fold the v bias through the output projection: softmax rows sum to
    # 1, so attn = softmax@v + bv and out = softmax@v@W_o + (bv@W_o + b_o)
    bo2 = np.asarray(b_o, np.float64) + np.asarray(
        b_qkv[2 * E:], np.float64) @ np.asarray(W_o, np.float64)
    bo = np.ascontiguousarray(np.broadcast_to(
        bo2.reshape(1, E), (P, E))).astype(_bf16)
    tri = np.triu(np.ones((P, P), np.float32)).astype(_bf16)
    maps = []
    for c in range(NCORES):
        o = c * HPC * D
        q_sl = slice(o, o + HPC * D)
        k_sl = slice(E + o, E + o + HPC * D)
        v_sl = slice(2 * E + o, 2 * E + o + HPC * D)
        wqk = np.concatenate(
            [W_qkv[:, q_sl] * 0.125, W_qkv[:, k_sl]], axis=1)
        maps.append({
            "xT": xT,
            "wqk": np.ascontiguousarray(
                wqk.reshape(EB, P, 2 * P).transpose(1, 0, 2)).astype(_bf16),
            "wv": np.ascontiguousarray(
                W_qkv[:, v_sl].reshape(EB, P, P).transpose(1, 0, 2)).astype(_bf16),
            "wo": wo,
            "bqk": np.stack([b_qkv[q_sl] * 0.125,
                             b_qkv[k_sl]], axis=1).astype(np.float32),
            "bo": bo,
            "tri": tri,
        })
    return maps


def kernel(x, W_qkv, b_qkv, W_o, b_o, mask):
    from concourse.bass_utils import run_bass_kernel_spmd

    if "nc" not in _cache:
        _cache["nc"] = _build()
    nc = _cache["nc"]
    maps = _in_maps(np.asarray(x, np.float32), np.asarray(W_qkv, np.float32),
                    np.asarray(b_qkv, np.float32), np.asarray(W_o, np.float32),
                    np.asarray(b_o, np.float32))
    res = run_bass_kernel_spmd(nc, maps, list(range(NCORES)))
    # rank r's out[st] is global 128-token block st*8 + r
    full = np.empty((SB, P, E), np.float32)
    for r in range(NCORES):
        full[r::NCORES] = res.results[r]["out"]
    return full.reshape(B, S, E).astype(np.float32)

